# revision 42
# baseline (speedup 1.0000x reference)
"""Multi-head causal attention (B=4, C=2048, E=1024, H=16, D=64) on 8 TRN2 cores.

Sharding: batch x head-group (4 x 2). Core c handles batch c//2 and heads
(c%2)*8 .. (c%2)*8+8.  Each core computes a partial output

    Y_c = Attn(x_b; heads hg) @ W_o[hg rows]        (shape [C, E])

and the host sums the two partials per batch (row-split W_o all-reduce done
host-side since outputs are gathered anyway).

v2 vs baseline (535 us):
  - fp16 matmul operands everywhere (PSUM accumulation stays fp32); host
    converts.  Enables FWL weight loads and 2x/4x DVE modes.
  - causality exploited at column granularity: for the 4 diagonal-straddle
    kk-tiles of each q-slice the S^T matmul, exp, and P@V matmul all run
    on the live columns [w:512) only (w = kkt*128 - j*512).  No gpsimd
    memsets of dead regions at all; dead e-columns are never written or
    read.  Only the 128-wide triangular straddle block needs a mask
    multiply (one DVE op covering both head-halves).
  - S pipeline at kk-tile granularity: s_ps tiles [128, 2(half), 512] f32
    (2 PSUM banks), 2 in flight, one exp call per kk-tile covering both
    halves.
  - softmax denominator rides row 64 of the P@V matmul (ones column in V);
    normalization = DVE reciprocal -> gpsimd partition_broadcast -> one
    fused DVE multiply straight out of PSUM into hdt (no PE broadcast
    matmul, frees a PSUM bank).
  - projections of slice j+1 and output-projection of slice j-1 are
    emitted as filler PE work interleaved into the attention kk-loops so
    the PE never idles long enough for HAM to re-throttle (the baseline
    ran most attention matmuls at the cold 1.2 GHz clock).
"""

import sys

if "/opt/trn_rl_repo" not in sys.path:
    sys.path.insert(0, "/opt/trn_rl_repo")

import math

import numpy as np

B, C, E, H, D = 4, 2048, 1024, 16, 64
NCORES = 8
P = 128
CS = 512  # q-slice width

USE_GPSIMD_BCAST = False  # InstPartitionBroadcast fails walrus codegen ("ISA wrong length")


def build_module(C=C, E=E, HL=H // 2, D=D, n_devices=NCORES):
    """Build the SPMD Bass module for one core's shard."""
    from contextlib import ExitStack

    import concourse.bass as bass
    import concourse.mybir as mybir
    import concourse.tile as tile

    F32 = mybir.dt.float32
    F16 = mybir.dt.float16
    FR = mybir.dt.float32r
    Exp = mybir.ActivationFunctionType.Exp
    MUL = mybir.AluOpType.mult

    ET = E // P          # e-tiles (8)
    JT = HL * D // P     # head pairs (4)
    NJ = C // CS         # q-slices (4)
    CT = C // P          # kk/c tiles (16)
    KPJ = CS // P        # kk-tiles per q-slice (4)
    scale = 1.0 / math.sqrt(D)

    nc = bass.Bass(
        "TRN2", target_bir_lowering=False, debug=False, num_devices=n_devices
    )

    xT = nc.dram_tensor("xT", [P, ET, C], F16, kind="ExternalInput").ap()
    wq_d = nc.dram_tensor("wq", [P, ET, HL * D], F16, kind="ExternalInput").ap()
    wk_d = nc.dram_tensor("wk", [P, ET, HL * D], F16, kind="ExternalInput").ap()
    wv_d = nc.dram_tensor("wv", [P, ET, HL * D], F16, kind="ExternalInput").ap()
    wo_d = nc.dram_tensor("wo", [P, JT, E], F16, kind="ExternalInput").ap()
    msk_d = nc.dram_tensor("msk", [P, 2, P], F16, kind="ExternalInput").ap()
    y_d = nc.dram_tensor("y", [CT, P, E], F16, kind="ExternalOutput").ap()

    with tile.TileContext(nc) as tc:
        with ExitStack() as ctx:
            pA = ctx.enter_context(tc.tile_pool(name="pA", bufs=1))
            pX = ctx.enter_context(tc.tile_pool(name="pX", bufs=2))
            pE = ctx.enter_context(tc.tile_pool(name="pE", bufs=4))
            pT = ctx.enter_context(tc.tile_pool(name="pT", bufs=3))
            psS = ctx.enter_context(tc.tile_pool(name="psS", bufs=2, space="PSUM"))
            psPV = ctx.enter_context(tc.tile_pool(name="psPV", bufs=2, space="PSUM"))
            psMM = ctx.enter_context(tc.tile_pool(name="psMM", bufs=2, space="PSUM"))

            # persistent SBUF residents
            qt = pA.tile([P, JT, C], F16, tag="qt")
            kt = pA.tile([P, JT, C], F16, tag="kt")
            v = pA.tile([P, CT, HL, D + 1], F16, tag="v")
            hdt = pA.tile([P, JT, C], F16, tag="hdt")
            wq = pA.tile([P, ET, HL * D], F16, tag="wq")
            wk = pA.tile([P, ET, HL * D], F16, tag="wk")
            wv = pA.tile([P, ET, HL * D], F16, tag="wv")
            wo = pA.tile([P, JT, E], F16, tag="wo")
            msk = pA.tile([P, 2, P], F16, tag="msk")
            ones = pA.tile([P, 64], F16, tag="ones")

            # slice-0 x and W_q stream in per-e-tile (split further across DMA
            # queues) so the first projection matmul starts after ~1/32 of the
            # data has landed instead of waiting for whole tensors
            # (finer sub-et chunking measured WORSE: 512B-per-partition DMA
            # lines run the queues at ~35% efficiency)
            xt0 = pX.tile([P, ET, CS], F16, tag="xt")
            EH = ET // 2
            for et in range(ET):
                nc.sync.dma_start(xt0[:, et, :], xT[:, et, 0:CS])
                nc.sync.dma_start(wq[:, et, :], wq_d[:, et, :])
                # wk/wv stream in halves between the wq chunks so the K/V
                # projection groups aren't starved behind whole tensors
                if et == 2:
                    nc.sync.dma_start(wk[:, 0:EH, :], wk_d[:, 0:EH, :])
                elif et == 5:
                    nc.sync.dma_start(wk[:, EH:, :], wk_d[:, EH:, :])
            nc.sync.dma_start(wv[:], wv_d)
            nc.sync.dma_start(wo[:], wo_d)
            nc.sync.dma_start(msk[:], msk_d)
            nc.vector.memset(ones[:], 1.0)
            nc.vector.memset(v[:, :, :, D : D + 1], 1.0)

            def act_reciprocal(out_ap, in_ap):
                """ACT-engine reciprocal. bass bans this for accuracy, but
                ~1e-3 relative accuracy is plenty for a softmax denominator
                (tolerance here is 2e-2), and it is ~5x faster than the DVE
                InstReciprocal which measured 3.3us/call and starved the PE."""
                sc = nc.scalar
                imm = lambda x: mybir.ImmediateValue(
                    dtype=mybir.dt.float32, value=x
                )
                return sc.add_instruction(
                    mybir.InstActivation(
                        name=sc.bass.get_next_instruction_name(),
                        func=mybir.ActivationFunctionType.Reciprocal,
                        ins=[sc.lower_ap(in_ap), imm(0.0), imm(1.0), imm(0.0)],
                        outs=[sc.lower_ap(out_ap)],
                    )
                )

            # ---------- projection emission (yields filler groups) ----------
            def proj_slice_groups(cs, xt=None):
                """Generator of thunks; each emits one PE accumulation group
                (8 et-matmuls + evict) for q/c-slice cs."""
                if xt is None:
                    xt = pX.tile([P, ET, CS], F16, tag="xt")
                    nc.sync.dma_start(xt[:], xT[:, :, cs * CS : (cs + 1) * CS])
                csl = slice(cs * CS, (cs + 1) * CS)

                def qk_group(w_sb, out_t, jt):
                    def emit():
                        ps = psMM.tile([P, CS], F32, tag="mm")
                        for et in range(ET):
                            nc.tensor.matmul(
                                ps[:],
                                w_sb[:, et, jt * P : (jt + 1) * P],
                                xt[:, et, :],
                                start=(et == 0),
                                stop=(et == ET - 1),
                            )
                        nc.vector.tensor_copy(out_t[:, jt, csl], ps[:])

                    return emit

                def v_group(c4):
                    def emit():
                        ct = cs * KPJ + c4
                        ps = psMM.tile([P, HL, D], F32, tag="mm")
                        for et in range(ET):
                            nc.tensor.matmul(
                                ps[:],
                                xt[:, et, c4 * P : (c4 + 1) * P],
                                wv[:, et, :],
                                start=(et == 0),
                                stop=(et == ET - 1),
                            )
                        nc.vector.tensor_copy(v[:, ct, :, 0:D], ps[:])

                    return emit

                for jt in range(JT):
                    yield qk_group(wq, qt, jt)
                for jt in range(JT):
                    yield qk_group(wk, kt, jt)
                for c4 in range(KPJ):
                    yield v_group(c4)

            # ---------- output projection emission ----------
            def outproj_slice_groups(j, tail=False):
                """Generator of thunks; each emits one (ct, fs) PE group.
                tail=True (last slice): evict via the then-idle ACT engine and
                assemble full [P, E] rows so the final y DMAs run at 2KB-line
                efficiency instead of draining ~8us after the last matmul."""
                FS = CS
                for c4 in range(KPJ):
                    ysb2 = (
                        pT.tile([P, E], F16, tag="ysb", name=f"ysb2_{j}_{c4}")
                        if tail
                        else None
                    )
                    for fs in range(E // FS):

                        def emit(c4=c4, fs=fs, ysb2=ysb2):
                            ct = j * KPJ + c4
                            fsl = slice(fs * FS, (fs + 1) * FS)
                            ps = psMM.tile([P, FS], F32, tag="mm")
                            for jt in range(JT):
                                nc.tensor.matmul(
                                    ps[:],
                                    hdt[:, jt, ct * P : (ct + 1) * P],
                                    wo[:, jt, fsl],
                                    start=(jt == 0),
                                    stop=(jt == JT - 1),
                                )
                            if ysb2 is not None:
                                nc.scalar.copy(ysb2[:, fsl], ps[:])
                                if fs == E // FS - 1:
                                    nc.sync.dma_start(y_d[ct, :, :], ysb2[:])
                            else:
                                ysb = pT.tile([P, FS], F16, tag="ysb")
                                nc.vector.tensor_copy(ysb[:], ps[:])
                                nc.sync.dma_start(y_d[ct, :, fsl], ysb[:])

                        yield emit

            # ---------- main fused loop ----------
            # slice-0 projections run up front (also HAM warm-up)
            for g in proj_slice_groups(0, xt=xt0):
                g()

            for j in range(NJ):
                jsl = slice(j * CS, (j + 1) * CS)
                nkt = (j + 1) * KPJ
                # filler work paced evenly over this slice's attention:
                # projections of slice j+1, outproj of slice j-1
                filler = []
                if j + 1 < NJ:
                    filler.extend(proj_slice_groups(j + 1))
                if j >= 1:
                    filler.extend(outproj_slice_groups(j - 1))
                # each g contributes nkt+3 tick() calls; matching the pace to
                # that keeps filler in reserve for the end-of-slice normalize
                # chains instead of exhausting it ~90% through the slice
                nticks = 4 * (nkt + 3)
                L = len(filler)
                fstate = [0, 0]  # ticks, emitted

                def tick(fstate=fstate, filler=filler, L=L, nticks=nticks):
                    fstate[0] += 1
                    want = min(L, fstate[0] * L // nticks)
                    while fstate[1] < want:
                        filler[fstate[1]]()
                        fstate[1] += 1

                for g in range(JT):
                    pv_ps = [
                        psPV.tile([D + 1, CS], F32, tag="pv", name=f"pv{h}")
                        for h in range(2)
                    ]
                    es = {}
                    for kkt in range(nkt):
                        w = kkt * P - j * CS
                        wc = max(w, 0)
                        ksl = slice(kkt * P, (kkt + 1) * P)
                        qsl = slice(j * CS + wc, (j + 1) * CS)
                        # S^T: both halves, live columns only
                        s_ps = psS.tile([P, 2, CS], F32, tag="s")
                        for half, base in ((0, 0), (1, 64)):
                            nc.tensor.matmul(
                                s_ps[:, half, wc:],
                                kt[base : base + 64, g, ksl],
                                qt[base : base + 64, g, qsl],
                                start=True,
                                stop=True,
                                tile_position=(base, 0),
                            )
                        # exp (scale folded), both halves in one ACT call
                        e = pE.tile([P, 2, CS], F16, tag="e")
                        nc.scalar.activation(
                            e[:, :, wc:], s_ps[:, :, wc:], Exp, scale=scale
                        )
                        # triangular mask on the 128-wide diagonal straddle
                        if w >= 0:
                            blk = e[:, :, wc : wc + P]
                            nc.vector.tensor_tensor(blk, blk, msk[:], MUL)
                        es[kkt] = (e, wc)
                        # P@V lags the S/exp pipeline by TWO kk-tiles: exp of
                        # a tile (~1.0us) is slower than the PE's S work
                        # (~0.8us), so at lag 1 every PV matmul waited ~0.2us
                        # on its exp.  e-tiles live in SBUF (pool depth 4) so
                        # the extra lag costs no PSUM.
                        if kkt >= 2:
                            pvt = kkt - 2
                            ep, wp = es.pop(pvt)
                            for half in range(2):
                                nc.tensor.matmul(
                                    pv_ps[half][:, wp:],
                                    v[:, pvt, 2 * g + half, :],
                                    ep[:, half, wp:],
                                    start=(pvt == 0),
                                    stop=False,
                                )
                        tick()
                    for pvt in (nkt - 2, nkt - 1):
                        ep, wp = es.pop(pvt)
                        for half in range(2):
                            nc.tensor.matmul(
                                pv_ps[half][:, wp:],
                                v[:, pvt, 2 * g + half, :],
                                ep[:, half, wp:],
                                start=(pvt == 0),
                                stop=(pvt == nkt - 1),
                            )
                    # keep the PE busy while the normalize chain (ACT recip +
                    # DMA broadcast + DVE multiply) runs off the critical path
                    tick()
                    # normalize: ACT reciprocal of the PSUM colsum row, DMA
                    # partition-broadcast, fused DVE multiply into hdt.
                    # No PE or psMM involvement at all.
                    for half in range(2):
                        # 1/d as exp(-ln d): ln and exp live in the SAME ACT
                        # table set (natural_log_exp_and_others), unlike
                        # Reciprocal, whose table alternation with Exp cost a
                        # 1.28us ACT table reload at every g boundary.  The
                        # ln(denom row) -> PE ones-matmul partition broadcast
                        # -> exp(-x) of the broadcast.  (0-stride broadcast
                        # APs are rejected by the DVE/DMA lowering, so the
                        # PE ones-matmul is the broadcast mechanism.)
                        lnr = pT.tile([D + 1, CS], F16, tag="rec")
                        nc.scalar.activation(
                            lnr[D : D + 1, :],
                            pv_ps[half][D : D + 1, :],
                            mybir.ActivationFunctionType.Ln,
                        )
                        bp = psMM.tile([64, CS], F32, tag="mm")
                        nc.tensor.matmul(
                            bp[:],
                            ones[64:65, :],
                            lnr[D : D + 1, :],
                            start=True,
                            stop=True,
                            tile_position=(64, 0),
                        )
                        bc = pT.tile([64, CS], F32, tag="bc")
                        nc.scalar.activation(bc[:], bp[:], Exp, scale=-1.0)
                        if half == 0:
                            nc.vector.tensor_tensor(
                                hdt[0:64, g, jsl], pv_ps[half][0:D, :], bc[:], MUL
                            )
                        else:
                            tmp = pT.tile([64, CS], F16, tag="tmp")
                            nc.vector.tensor_tensor(
                                tmp[:], pv_ps[half][0:D, :], bc[:], MUL
                            )
                            nc.sync.dma_start(hdt[64:128, g, jsl], tmp[:])
                    tick()
                    tick()
                # all filler must land inside this slice (attention of slice
                # j+1 needs slice j+1's projections complete)
                while fstate[1] < L:
                    filler[fstate[1]]()
                    fstate[1] += 1
            for g in outproj_slice_groups(NJ - 1, tail=True):
                g()
    return nc


def _split_waits_json(bir_json_bytes):
    """TRN2 TPB instructions have one sync-wait slot and this walrus build
    refuses to split multi-wait instructions, so hoist all but the last wait
    onto preceding wait-only EventSemaphore instructions (same engine,
    executed in order -> semantically identical)."""
    import json

    d = json.loads(bir_json_bytes)
    n = 0
    for fn in d["functions"]:
        for blk in fn["blocks"]:
            out = []
            for inst in blk["instructions"]:
                si = inst.get("sync_info")
                waits = (si or {}).get("on_wait") or []
                if len(waits) > 1:
                    for w in waits[:-1]:
                        n += 1
                        out.append(
                            {
                                "debug": inst.get("debug", 0),
                                "engine": inst["engine"],
                                "ins": [],
                                "name": f"wsplit-{n}",
                                "opcode": "EventSemaphore",
                                "outs": [],
                                "sync_info": {"on_update": [], "on_wait": [w]},
                            }
                        )
                    si["on_wait"] = [waits[-1]]
                out.append(inst)
            blk["instructions"] = out
    return json.dumps(d).encode()


def _striped(a, p=P):
    """[K, N] with K = kt*p + i  ->  contiguous [p, K//p, N]."""
    k, n = a.shape
    return np.ascontiguousarray(a.reshape(k // p, p, n).transpose(1, 0, 2))


def prep_core_inputs(x_b, wq_s, wk_s, wv_s, wo_s):
    """Host-side layout prep for one core. x_b [C,E], w*_s column/row slices.
    Everything fp16."""
    tri = np.triu(np.ones((P, P), dtype=np.float16))  # keep where q >= kk
    msk = np.ascontiguousarray(np.stack([tri, tri], axis=1))  # [P, 2, P]
    f16 = np.float16
    return {
        "xT": _striped(np.ascontiguousarray(x_b.T)).astype(f16),
        "wq": _striped(wq_s).astype(f16),
        "wk": _striped(wk_s).astype(f16),
        "wv": _striped(wv_s).astype(f16),
        "wo": _striped(wo_s).astype(f16),
        "msk": msk,
    }


_module_cache = {}


def _enable_ldw_opt():
    """walrus runs with --enable-ldw-opt=false by default in this harness;
    enabling it overlaps LDWEIGHTS with matmuls (~40ns/matmul here)."""
    import os

    if not os.environ.get("LDW_OPT"):
        return
    import concourse.bass_utils as bu

    if getattr(bu, "_ldw_opt_patched", False):
        return
    orig = bu.run_command

    def patched(argv, **kw):
        argv = [
            a.replace("--enable-ldw-opt=false", "--enable-ldw-opt=true")
            for a in argv
        ]
        return orig(argv, **kw)

    bu.run_command = patched
    bu._ldw_opt_patched = True


def kernel(x, W_q, W_k, W_v, W_o):
    from concourse.bass_utils import run_bass_kernel_spmd

    _enable_ldw_opt()

    x = np.asarray(x, dtype=np.float32)
    W_q = np.asarray(W_q, dtype=np.float32)
    W_k = np.asarray(W_k, dtype=np.float32)
    W_v = np.asarray(W_v, dtype=np.float32)
    W_o = np.asarray(W_o, dtype=np.float32)

    HD2 = H * D // 2  # columns per head-group (512)
    in_maps = []
    for core in range(NCORES):
        b, hg = core // 2, core % 2
        cols = slice(hg * HD2, (hg + 1) * HD2)
        in_maps.append(
            prep_core_inputs(
                x[b], W_q[:, cols], W_k[:, cols], W_v[:, cols], W_o[cols, :]
            )
        )

    if "nc" not in _module_cache:
        nc = build_module()
        fixed = _split_waits_json(nc.to_json_bytes())
        nc.to_json_bytes = lambda: fixed
        _module_cache["nc"] = nc
    nc = _module_cache["nc"]

    res = run_bass_kernel_spmd(nc, in_maps, core_ids=list(range(NCORES)))
    _module_cache["last_res"] = res
    out = np.empty((B, C, E), dtype=np.float32)
    for b in range(B):
        ya = res.results[2 * b]["y"].reshape(C, E).astype(np.float32)
        yb = res.results[2 * b + 1]["y"].reshape(C, E).astype(np.float32)
        out[b] = ya + yb
    return out


if __name__ == "__main__":
    rng = np.random.default_rng(0)
    ins = {
        "x": rng.standard_normal((B, C, E), dtype=np.float32),
        "W_q": rng.standard_normal((E, H * D), dtype=np.float32) * 0.02,
        "W_k": rng.standard_normal((E, H * D), dtype=np.float32) * 0.02,
        "W_v": rng.standard_normal((E, H * D), dtype=np.float32) * 0.02,
        "W_o": rng.standard_normal((H * D, E), dtype=np.float32) * 0.02,
    }
    out = kernel(**ins)
    print("kernel ran, out shape", out.shape, "mean", out.mean())
